# revision 22
# baseline (speedup 1.0000x reference)
import numpy as np
from ml_dtypes import bfloat16, float8_e3m4

import concourse.bass as bass
import concourse.mybir as mybir
import concourse.tile as tile
from concourse import bacc
from concourse.bass_utils import run_bass_kernel_spmd

B, C_IN, H, W = 32, 32, 64, 64
C_OUT, OH, OW, KK = 64, 62, 62, 3
N_CORES = 8
ROWS = 8
HALF = 31
XH = ROWS + 2
KP = 97
HZ = OW * B
QZ = HALF * C_OUT
F32 = mybir.dt.float32
BF16 = mybir.dt.bfloat16
FP8 = mybir.dt.float8e3

_NC_CACHE = {}


def _build_nc():
    nc = bacc.Bacc(
        "TRN2",
        target_bir_lowering=False,
        debug=False,
        enable_asserts=False,
        num_devices=N_CORES,
    )
    x_d = nc.dram_tensor("x", [128, XH, OW, B], FP8, kind="ExternalInput").ap()
    w_d = nc.dram_tensor(
        "w", [ROWS, 2, 128, 3, HALF, C_OUT], FP8, kind="ExternalInput"
    ).ap()
    NG = 8
    o_d = nc.dram_tensor(
        "out", [ROWS, 2, 128, NG * C_OUT], BF16, kind="ExternalOutput"
    ).ap()

    with tile.TileContext(nc) as tc:
        with (
            tc.tile_pool(name="xpool", bufs=1) as xpool,
            tc.tile_pool(name="wpool", bufs=8) as wpool,
            tc.tile_pool(name="opool", bufs=2) as opool,
            tc.tile_pool(name="pspool", bufs=6, space="PSUM") as pspool,
            tc.tile_pool(name="wupool", bufs=2, space="PSUM") as wupool,
        ):
            x3 = xpool.tile([128, XH * HZ], FP8)
            xsrc = x_d.rearrange("p h w b -> p (h w b)")
            xrows = {0: (3, 4), 1: (4, 5), 3: (5, 6), 5: (6, 7), 7: (7, 8),
                     9: (8, 9), 11: (9, XH)}

            def load_x(h0, h1, eng):
                eng.dma_start(
                    out=x3[:, h0 * HZ : h1 * HZ], in_=xsrc[:, h0 * HZ : h1 * HZ]
                )

            load_x(0, 1, nc.gpsimd)

            wu = xpool.tile([128, 544], BF16)
            nc.vector.memset(wu, 0.0)
            for i in range(8):
                wps = wupool.tile([128, 512], F32, tag="wps")
                li = i % 4
                nc.tensor.matmul(
                    wps[32 * li : 32 * li + 32, :],
                    wu[0:KP, 0:32],
                    wu[0:KP, 32:544],
                    start=True,
                    stop=True,
                    tile_position=(0, 32 * li),
                    skip_group_check=True,
                )

            for row in range(ROWS):
                for half in range(2):
                    strip = row * 2 + half
                    wt = wpool.tile([128, 3 * QZ], FP8, tag="wt")
                    weng = nc.gpsimd
                    if strip in xrows:
                        load_x(*xrows[strip], nc.gpsimd)
                    wsrc = w_d[row, half].rearrange("p q l o -> p (q l o)")
                    if strip in (0, 15):
                        for f0 in range(3):
                            weng.dma_start(
                                out=wt[:, f0 * QZ : (f0 + 1) * QZ],
                                in_=wsrc[:, f0 * QZ : (f0 + 1) * QZ],
                            )
                            if strip == 0 and f0 < 2:
                                load_x(f0 + 1, f0 + 2, nc.gpsimd)
                    else:
                        weng.dma_start(out=wt, in_=wsrc)
                    ot = opool.tile([128, NG * C_OUT], BF16, tag="ot")
                    ps = pspool.tile([128, NG * C_OUT], F32, tag="ps")
                    for g in range(NG):
                        gn = min(4, HALF - g * 4)
                        for q in range(3):
                            for li in range(4):
                                if li >= gn and q > 0:
                                    continue
                                eff = min(li, gn - 1)
                                lo = g * 4 + eff
                                ow = half * HALF + lo
                                nc.tensor.matmul(
                                    ps[32 * li : 32 * li + 32, g * C_OUT : (g + 1) * C_OUT],
                                    x3[0:KP, (row + q) * HZ + ow * B : (row + q) * HZ + ow * B + B],
                                    wt[0:KP, q * QZ + lo * C_OUT : q * QZ + lo * C_OUT + C_OUT],
                                    start=(q == 0),
                                    stop=(q == 2) or (li >= gn),
                                    tile_position=(0, 32 * li),
                                    skip_group_check=True,
                                )
                    nc.vector.tensor_copy(out=ot, in_=ps)
                    oeng = nc.gpsimd if strip >= 14 else nc.scalar
                    oeng.dma_start(out=o_d[row, half], in_=ot)

    nc.compile()
    return nc


def get_nc():
    if "nc" not in _NC_CACHE:
        _NC_CACHE["nc"] = _build_nc()
    return _NC_CACHE["nc"]


def prep_inputs(x, weight, bias):
    x = np.asarray(x, dtype=np.float32)
    weight = np.asarray(weight, dtype=np.float32)
    bias = np.asarray(bias, dtype=np.float32)

    wp = np.zeros((N_CORES * ROWS, 128, 3, OW, C_OUT), np.float32)
    wp[:OH, :96] = weight.transpose(1, 5, 3, 4, 2, 0).reshape(OH, 96, 3, OW, C_OUT)
    wp[:OH, 96, 2] = bias.transpose(1, 2, 0)
    wp = wp.astype(float8_e3m4)
    wp = np.ascontiguousarray(
        wp.reshape(N_CORES * ROWS, 128, 3, 2, HALF, C_OUT).transpose(0, 3, 1, 2, 4, 5)
    )

    xp = np.zeros((B, C_IN, N_CORES * ROWS + 2, W), np.float32)
    xp[:, :, :H] = x
    xt = xp.transpose(1, 2, 3, 0).astype(float8_e3m4)

    in_maps = []
    for c in range(N_CORES):
        r0 = c * ROWS
        xc = xt[:, r0 : r0 + XH]
        xsh = np.zeros((128, XH, OW, B), float8_e3m4)
        for kj in range(KK):
            xsh[kj * 32 : kj * 32 + 32] = xc[:, :, kj : kj + OW, :]
        xsh[96] = 1.0
        in_maps.append(
            {
                "x": xsh,
                "w": np.ascontiguousarray(wp[r0 : r0 + ROWS]),
            }
        )
    return in_maps


def gather_output(results):
    out = np.empty((B, C_OUT, OH, OW), np.float32)
    for c in range(N_CORES):
        oc = np.asarray(results[c]["out"], dtype=np.float32)
        v = oc.reshape(ROWS, 2, 4, B, 8, C_OUT)
        arr = v.transpose(3, 5, 0, 1, 4, 2).reshape(B, C_OUT, ROWS, 2, 32)
        arr = arr[:, :, :, :, :HALF].reshape(B, C_OUT, ROWS, OW)
        r0 = c * ROWS
        rows = min(ROWS, OH - r0)
        out[:, :, r0 : r0 + rows, :] = arr[:, :, :rows, :]
    return out


def run(inputs, **kw):
    nc = get_nc()
    in_maps = prep_inputs(inputs["x"], inputs["weight"], inputs["bias"])
    res = run_bass_kernel_spmd(nc, in_maps, core_ids=list(range(N_CORES)), **kw)
    return gather_output(res.results), res


def kernel(x, weight, bias):
    out, _ = run({"x": x, "weight": weight, "bias": bias})
    return out
